# revision 1
# baseline (speedup 1.0000x reference)
"""GCN encoder (kNN softmax message passing, 3 layers) on 8 Trainium2 cores.

Contract: kernel(**inputs) takes FULL numpy inputs (as produced by
setup_inputs()) and returns the FULL (B, N, H) float32 output.

Sharding: data-parallel over batch B=16 -> 2 samples per core on 8 cores.

Per-core device algorithm (per sample):
  1. Selection: for each 128-row tile of dist, negate (ACT), top-8 values
     (DVE max8) + their positions (DVE max_index), softmax over top-5 (tiny),
     scatter the 5 weights into a dense fp16 row via GPSIMD local_scatter.
  2. Repartition: SBUF-source dma_gather(transpose=True) with an identity
     index table rewrites w (node-on-partition) to wT (neighbor-on-partition).
  3. Layers x3: aggT = h16^T-contract wT (PE fp16 matmuls), delta^T =
     relu(W @ aggT + b) (PE fp32 + ACT), transpose + identity-matmul residual
     into PSUM (PE), layernorm via ACT Square + DVE 3D reduces + per-tile ACT
     normalize.
"""

from contextlib import ExitStack

import numpy as np

import concourse.bacc as bacc
import concourse.tile as tile
from concourse import mybir
from concourse.bass_utils import run_bass_kernel_spmd

F32 = mybir.dt.float32
F16 = mybir.dt.float16
U16 = mybir.dt.uint16
I16 = mybir.dt.int16

B, N, H, L, K = 16, 2000, 128, 3, 5
N_CORES = 8
SPC = B // N_CORES          # samples per core
NT = (N + 127) // 128       # 16 node tiles (last has 80 rows)
NP = NT * 128               # 2048 padded nodes
LN_EPS = 1e-5
AF = mybir.ActivationFunctionType


def _build_program(trivial_affine, compile=True):
    nc = bacc.Bacc("TRN2", target_bir_lowering=False, debug=False)

    dist = nc.dram_tensor("dist", [SPC, N, N], F32, kind="ExternalInput").ap()
    emb = nc.dram_tensor("emb", [SPC, N, H], F32, kind="ExternalInput").ap()
    wsT = nc.dram_tensor("wsT", [L, H, H], F32, kind="ExternalInput").ap()
    bsin = nc.dram_tensor("bs", [L, H, 1], F32, kind="ExternalInput").ap()
    ident = nc.dram_tensor("ident", [H, H], F32, kind="ExternalInput").ap()
    gidx = nc.dram_tensor("gidx", [128, NP // 16], I16, kind="ExternalInput").ap()
    out = nc.dram_tensor("out", [SPC, N, H], F32, kind="ExternalOutput").ap()
    if not trivial_affine:
        grep = nc.dram_tensor("grep", [L, H, H], F32, kind="ExternalInput").ap()
        brep = nc.dram_tensor("brep", [L, H, H], F32, kind="ExternalInput").ap()

    with tile.TileContext(nc) as tc, ExitStack() as ctx:
        big = ctx.enter_context(tc.tile_pool(name="big", bufs=1))
        dpool = ctx.enter_context(tc.tile_pool(name="dist", bufs=2))
        sel = ctx.enter_context(tc.tile_pool(name="sel", bufs=3))
        ln = ctx.enter_context(tc.tile_pool(name="ln", bufs=2))
        ps_agg = ctx.enter_context(tc.tile_pool(name="ps_agg", bufs=2, space="PSUM"))
        ps_d = ctx.enter_context(tc.tile_pool(name="ps_d", bufs=2, space="PSUM"))
        ps_r = ctx.enter_context(tc.tile_pool(name="ps_r", bufs=1, space="PSUM"))

        # persistent state
        w_nat = big.tile([128, NT, NP], F16, tag="w_nat")

        h_nat = big.tile([128, NT, H], F32, tag="h_nat")
        h16 = big.tile([128, NT, H], F16, tag="h16")
        aggT_sb = big.tile([128, NP], F32, tag="aggT_sb")
        deltaT_sb = big.tile([128, NP], F32, tag="deltaT_sb")
        sq_sb = big.tile([128, NP], F32, tag="sq_sb")
        wsT_sb = big.tile([128, L * H], F32, tag="wsT_sb")
        bs_sb = big.tile([128, L], F32, tag="bs_sb")
        ident_sb = big.tile([128, H], F32, tag="ident_sb")
        gidx_sb = big.tile([128, NP // 16], I16, tag="gidx_sb")
        if not trivial_affine:
            grep_sb = big.tile([128, L * H], F32, tag="grep_sb")
            brep_sb = big.tile([128, L * H], F32, tag="brep_sb")

        # constants / weights
        for l in range(L):
            nc.sync.dma_start(wsT_sb[:, l * H:(l + 1) * H], wsT[l, :, :])
            nc.sync.dma_start(bs_sb[:, l:l + 1], bsin[l, :, :])
            if not trivial_affine:
                nc.sync.dma_start(grep_sb[:, l * H:(l + 1) * H], grep[l, :, :])
                nc.sync.dma_start(brep_sb[:, l * H:(l + 1) * H], brep[l, :, :])
        nc.sync.dma_start(ident_sb[:], ident[:, :])
        nc.sync.dma_start(gidx_sb[:], gidx[:, :])
        eps_sb = big.tile([128, 1], F32, tag="eps_sb")
        nc.vector.memset(eps_sb[:, :], LN_EPS)
        # zero the m-padding of every w row once; scatters never touch it
        if NP > N:
            nc.vector.memset(w_nat[:, :, N:NP], 0.0)

        CHUNK = min(512, NP)
        NCH = NP // CHUNK
        GIDX_PC = CHUNK // 16  # idx-table columns per gather chunk
        wT = big.tile([128, NCH, NT, CHUNK], F16, tag="wT")
        for s in range(SPC):
            # ---- selection + scatter -------------------------------------
            for t in range(NT):
                r0 = t * 128
                pp = min(128, N - r0)
                dt_ = dpool.tile([128, N], F32, tag="dt")
                nc.sync.dma_start(dt_[:pp, :], dist[s, r0:r0 + pp, :])
                # in-place negate: nd = -d
                nc.scalar.activation(dt_[:pp, :], dt_[:pp, :], AF.Copy,
                                     bias=0.0, scale=-1.0)
                m8 = sel.tile([128, 8], F32, tag="m8")
                nc.vector.max(m8[:pp, :], dt_[:pp, :])
                i16t = sel.tile([128, 8], U16, tag="i16")
                nc.vector.max_index(i16t[:pp, :], m8[:pp, :], dt_[:pp, :])
                # slot 5 becomes the local_scatter "ignore" index (-1)
                nc.vector.memset(i16t[:pp, 5:6], 65535)
                negm0 = sel.tile([128, 1], F32, tag="negm0")
                nc.vector.tensor_scalar_mul(negm0[:pp, :], m8[:pp, 0:1], -1.0)
                e5 = sel.tile([128, 5], F32, tag="e5")
                nc.scalar.activation(e5[:pp, :], m8[:pp, 0:5], AF.Exp,
                                     bias=negm0[:pp, :], scale=1.0)
                s5 = sel.tile([128, 1], F32, tag="s5")
                nc.vector.tensor_reduce(s5[:pp, :], e5[:pp, :],
                                        axis=mybir.AxisListType.X,
                                        op=mybir.AluOpType.add)
                r5 = sel.tile([128, 1], F32, tag="r5")
                nc.vector.reciprocal(r5[:pp, :], s5[:pp, :])
                w6 = sel.tile([128, 6], F16, tag="w6")
                nc.vector.tensor_scalar(w6[:pp, 0:5], e5[:pp, :], r5[:pp, :],
                                        None, mybir.AluOpType.mult)
                nc.vector.memset(w6[:pp, 5:6], 0.0)
                nc.gpsimd.local_scatter(
                    w_nat[0:pp, t, 0:N], w6[:pp, :],
                    i16t[:pp, 0:6].bitcast(I16),
                    channels=(pp + 15) // 16 * 16, num_elems=N, num_idxs=6)

            # ---- w -> wT repartition via transposing SBUF gathers --------
            # chunked (512 rows per call) to bound per-call SWDGE FIFO usage
            for j in range(NCH):
                nc.gpsimd.dma_gather(
                    out_ap=wT[:, j, :, :],
                    in_ap=w_nat[:, :, :],
                    idxs_ap=gidx_sb[:, j * GIDX_PC:(j + 1) * GIDX_PC],
                    num_idxs=CHUNK,
                    num_idxs_reg=CHUNK,
                    elem_size=NP,
                    transpose=True,
                    sbuf_tokens_per_rank=128,
                    sbuf_free_dim_per_rank=NP * 2,
                    queue_num=0,
                )

            # ---- h0 = emb ------------------------------------------------
            for t in range(NT):
                r0 = t * 128
                pp = min(128, N - r0)
                if pp < 128:
                    # legal partition bases are multiples of 32; zero the
                    # tail region first, the DMA then fills the valid rows
                    base = pp // 32 * 32
                    nc.vector.memset(h_nat[base:128, t, :], 0.0)
                nc.sync.dma_start(h_nat[:pp, t, :], emb[s, r0:r0 + pp, :])
            nc.scalar.copy(h16[:, :, :], h_nat[:, :, :])

            # ---- layers --------------------------------------------------
            for l in range(L):
                # aggT[hd, n] = sum_m h[m, hd] * w[n, m]
                for ch in range(NCH):
                    ps = ps_agg.tile([128, CHUNK], F32, tag="aggT")
                    for c in range(NT):
                        nc.tensor.matmul(
                            ps[:, :], lhsT=h16[:, c, :],
                            rhs=wT[:, ch, c, :],
                            start=(c == 0), stop=(c == NT - 1))
                    nc.scalar.copy(aggT_sb[:, ch * CHUNK:(ch + 1) * CHUNK], ps[:, :])
                # deltaT = relu(W @ aggT + b)
                for ch in range(NCH):
                    sl = slice(ch * CHUNK, (ch + 1) * CHUNK)
                    psd = ps_d.tile([128, CHUNK], F32, tag="deltaT")
                    nc.tensor.matmul(psd[:, :], lhsT=wsT_sb[:, l * H:(l + 1) * H],
                                     rhs=aggT_sb[:, sl], start=True, stop=True)
                    nc.scalar.activation(deltaT_sb[:, sl], psd[:, :], AF.Relu,
                                         bias=bs_sb[:, l:l + 1], scale=1.0)
                # r = delta^T^T + h  (transpose + identity-matmul residual).
                # PSUM start zeroes a whole 2KB zero-region, so flags are
                # grouped per bank: first matmul in a bank starts, last stops;
                # same-bank WAW deps keep emission order on the PE stream.
                r = ps_r.tile([128, NP], F32, tag="r")
                SPB = max(1, 512 // 128)  # 128-col slices per 2KB bank
                for g in range((NT + SPB - 1) // SPB):
                    ts_ = list(range(g * SPB, min((g + 1) * SPB, NT)))
                    for j, t in enumerate(ts_):
                        sl = slice(t * 128, (t + 1) * 128)
                        nc.tensor.matmul(r[:, sl], lhsT=deltaT_sb[:, sl],
                                         rhs=ident_sb[:, :], is_transpose=True,
                                         start=(j == 0), stop=False)
                    for j, t in enumerate(ts_):
                        sl = slice(t * 128, (t + 1) * 128)
                        nc.tensor.matmul(r[:, sl], lhsT=ident_sb[:, :],
                                         rhs=h_nat[:, t, :], start=False,
                                         stop=(j == len(ts_) - 1))
                # layernorm stats
                nc.scalar.square(sq_sb[:, :], r[:, :])
                sr = ln.tile([128, NT], F32, tag="sr")
                nc.vector.tensor_reduce(
                    sr[:, :], r[:, :].rearrange("p (t h) -> p t h", h=H),
                    axis=mybir.AxisListType.X, op=mybir.AluOpType.add)
                ssq = ln.tile([128, NT], F32, tag="ssq")
                nc.vector.tensor_reduce(
                    ssq[:, :], sq_sb[:, :].rearrange("p (t h) -> p t h", h=H),
                    axis=mybir.AxisListType.X, op=mybir.AluOpType.add)
                mu = ln.tile([128, NT], F32, tag="mu")
                nc.vector.tensor_scalar_mul(mu[:, :], sr[:, :], 1.0 / H)
                var = ln.tile([128, NT], F32, tag="var")
                # var = E[x^2] - mu^2  (+eps folded into sqrt bias)
                nc.vector.tensor_scalar_mul(var[:, :], ssq[:, :], 1.0 / H)
                musq = ln.tile([128, NT], F32, tag="musq")
                nc.vector.tensor_tensor(musq[:, :], mu[:, :], mu[:, :],
                                        mybir.AluOpType.mult)
                nc.vector.tensor_tensor(var[:, :], var[:, :], musq[:, :],
                                        mybir.AluOpType.subtract)
                sd = ln.tile([128, NT], F32, tag="sd")
                nc.scalar.activation(sd[:, :], var[:, :], AF.Sqrt,
                                     bias=eps_sb[:, :], scale=1.0)
                sinv = ln.tile([128, NT], F32, tag="sinv")
                nc.vector.reciprocal(sinv[:, :], sd[:, :])
                negmus = ln.tile([128, NT], F32, tag="negmus")
                nc.vector.tensor_tensor(negmus[:, :], mu[:, :], sinv[:, :],
                                        mybir.AluOpType.mult)
                nc.vector.tensor_scalar_mul(negmus[:, :], negmus[:, :], -1.0)
                # normalize: h = (r - mu) * sinv  [* gamma + beta]
                for t in range(NT):
                    sl = slice(t * 128, (t + 1) * 128)
                    if trivial_affine:
                        nc.scalar.activation(h_nat[:, t, :], r[:, sl],
                                             AF.Identity,
                                             bias=negmus[:, t:t + 1],
                                             scale=sinv[:, t:t + 1])
                    else:
                        nc.scalar.activation(sq_sb[:, sl], r[:, sl],
                                             AF.Identity,
                                             bias=negmus[:, t:t + 1],
                                             scale=sinv[:, t:t + 1])
                        nc.vector.tensor_tensor(
                            sq_sb[:, sl], sq_sb[:, sl],
                            grep_sb[:, l * H:(l + 1) * H],
                            mybir.AluOpType.mult)
                        nc.vector.tensor_tensor(
                            h_nat[:, t, :], sq_sb[:, sl],
                            brep_sb[:, l * H:(l + 1) * H],
                            mybir.AluOpType.add)
                if l < L - 1:
                    nc.scalar.copy(h16[:, :, :], h_nat[:, :, :])

            # ---- store ---------------------------------------------------
            for t in range(NT):
                r0 = t * 128
                pp = min(128, N - r0)
                nc.sync.dma_start(out[s, r0:r0 + pp, :], h_nat[:pp, t, :])

    if compile:
        nc.compile()
    return nc


def _gather_idx_table():
    """Index i of the gather lives at (partition i%16 + 16*core, free i//16),
    replicated across the 8 gpsimd cores; value = flat w_nat row of node i."""
    vals = np.arange(NP, dtype=np.int16)
    vals[N:] = 0
    tab = np.zeros((128, NP // 16), dtype=np.int16)
    for g in range(8):
        tab[g * 16:(g + 1) * 16, :] = vals.reshape(NP // 16, 16).T
    return tab


_CACHE = {}


def kernel(node_emb, dist_matrix, Ws, bs, gammas, betas):
    node_emb = np.ascontiguousarray(np.asarray(node_emb, dtype=np.float32))
    dist_matrix = np.ascontiguousarray(np.asarray(dist_matrix, dtype=np.float32))
    Ws = np.asarray(Ws, dtype=np.float32)
    bs = np.asarray(bs, dtype=np.float32)
    gammas = np.asarray(gammas, dtype=np.float32)
    betas = np.asarray(betas, dtype=np.float32)

    trivial = bool(np.all(gammas == 1.0) and np.all(betas == 0.0))
    key = ("prog", trivial)
    if key not in _CACHE:
        _CACHE[key] = _build_program(trivial)
    nc = _CACHE[key]

    wsT = np.ascontiguousarray(np.transpose(Ws, (0, 2, 1)))
    bs3 = np.ascontiguousarray(bs[:, :, None])
    ident = np.eye(H, dtype=np.float32)
    gtab = _gather_idx_table()

    in_maps = []
    for c in range(N_CORES):
        m = {
            "dist": dist_matrix[c * SPC:(c + 1) * SPC],
            "emb": node_emb[c * SPC:(c + 1) * SPC],
            "wsT": wsT,
            "bs": bs3,
            "ident": ident,
            "gidx": gtab,
        }
        if not trivial:
            m["grep"] = np.ascontiguousarray(
                np.broadcast_to(gammas[:, None, :], (L, H, H)))
            m["brep"] = np.ascontiguousarray(
                np.broadcast_to(betas[:, None, :], (L, H, H)))
        in_maps.append(m)

    res = run_bass_kernel_spmd(nc, in_maps, list(range(N_CORES)))
    kernel.last_results = res
    out = np.concatenate([res.results[c]["out"] for c in range(N_CORES)], axis=0)
    return out



# revision 8
# speedup vs baseline: 1.2192x; 1.2192x over previous
"""GCN encoder (kNN softmax message passing, 3 layers) on 8 Trainium2 cores.

Contract: kernel(**inputs) takes FULL numpy inputs (as produced by
setup_inputs()) and returns the FULL (B, N, H) float32 output.

Sharding: data-parallel over batch B=16 -> 2 samples per core on 8 cores.

Per-core device algorithm (per sample), structured so sample s+1's
selection pipeline overlaps sample s's layer compute:
  1. Selection (per 128-row dist tile): negate (ACT), top-8 values (DVE
     max8) + positions (DVE max_index), softmax over top-5 with the sum
     from ACT's accumulator, scatter 6 (value, index) pairs into a dense
     fp16 row via GPSIMD local_scatter.  w rows are built in 4-tile
     "groups" ([128, 4, NP]) from a small ring so gathers free them fast.
  2. Repartition: per group, an SBUF-source dma_gather(transpose=True)
     with a local identity index table rewrites the group's 512 node rows
     into wT chunk tiles (neighbor-on-partition).  Chunk tiles are
     separate tags so next sample's gathers only WAR on the chunk's last
     matmul reader.
  3. Layers x3 (all-fp16 PE operands): aggT = h16^T-contract wT,
     deltaT = relu(W @ aggT + b), transpose + identity-matmul residual
     into PSUM, layernorm stats via ACT square + DVE reduces, normalize
     via engine-balanced (nc.any) tensor_scalar into h16 (fp16) or the
     f32 output buffer on the last layer.
"""

from contextlib import ExitStack

import numpy as np

import concourse.bacc as bacc
import concourse.tile as tile
from concourse import mybir
from concourse.bass_utils import run_bass_kernel_spmd

F32 = mybir.dt.float32
F16 = mybir.dt.float16
U16 = mybir.dt.uint16
I16 = mybir.dt.int16

B, N, H, L, K = 16, 2000, 128, 3, 5
N_CORES = 8
SPC = B // N_CORES          # samples per core
NT = (N + 127) // 128       # 16 node tiles (last has 80 rows)
NP = NT * 128               # 2048 padded nodes
NG = 4                      # node tiles per scatter/gather group
NGRP = NT // NG             # 4 groups per sample
CHUNK = 512                 # gather chunk = nodes per wT chunk tile
LN_EPS = 1e-5
AF = mybir.ActivationFunctionType
OP = mybir.AluOpType


def _build_program(trivial_affine, compile=True):
    nc = bacc.Bacc("TRN2", target_bir_lowering=False, debug=False)

    dist = nc.dram_tensor("dist", [SPC, N, N], F32, kind="ExternalInput").ap()
    emb = nc.dram_tensor("emb", [SPC, N, H], F32, kind="ExternalInput").ap()
    wsT = nc.dram_tensor("wsT", [L, H, H], F16, kind="ExternalInput").ap()
    bsin = nc.dram_tensor("bs", [L, H, 1], F32, kind="ExternalInput").ap()
    ident = nc.dram_tensor("ident", [H, H], F16, kind="ExternalInput").ap()
    identf = nc.dram_tensor("identf", [H, H], F32, kind="ExternalInput").ap()
    gidx = nc.dram_tensor("gidx", [128, CHUNK // 16], I16, kind="ExternalInput").ap()
    out = nc.dram_tensor("out", [SPC, N, H], F32, kind="ExternalOutput").ap()
    if not trivial_affine:
        grep = nc.dram_tensor("grep", [L, H, H], F32, kind="ExternalInput").ap()
        brep = nc.dram_tensor("brep", [L, H, H], F32, kind="ExternalInput").ap()

    with tile.TileContext(nc) as tc, ExitStack() as ctx:
        big = ctx.enter_context(tc.tile_pool(name="big", bufs=1))
        dpool = ctx.enter_context(tc.tile_pool(name="dist", bufs=3))
        sel = ctx.enter_context(tc.tile_pool(name="sel", bufs=4))
        wnp = ctx.enter_context(tc.tile_pool(name="wn", bufs=3))
        wtp = ctx.enter_context(tc.tile_pool(name="wt", bufs=1))
        hp = ctx.enter_context(tc.tile_pool(name="h", bufs=2))
        agp = ctx.enter_context(tc.tile_pool(name="ag", bufs=2))
        ln = ctx.enter_context(tc.tile_pool(name="ln", bufs=4))
        ps_agg = ctx.enter_context(tc.tile_pool(name="ps_agg", bufs=2, space="PSUM"))
        ps_d = ctx.enter_context(tc.tile_pool(name="ps_d", bufs=2, space="PSUM"))
        ps_r = ctx.enter_context(tc.tile_pool(name="ps_r", bufs=1, space="PSUM"))

        # constants
        wsT_sb = big.tile([128, L * H], F16, tag="wsT_sb")
        bs_sb = big.tile([128, L], F32, tag="bs_sb")
        ident_sb = big.tile([128, H], F16, tag="ident_sb")
        identf_sb = big.tile([128, H], F32, tag="identf_sb")
        gidx_sb = big.tile([128, CHUNK // 16], I16, tag="gidx_sb")
        eps_sb = big.tile([128, 1], F32, tag="eps_sb")
        if not trivial_affine:
            grep_sb = big.tile([128, L * H], F32, tag="grep_sb")
            brep_sb = big.tile([128, L * H], F32, tag="brep_sb")
        for l in range(L):
            nc.sync.dma_start(wsT_sb[:, l * H:(l + 1) * H], wsT[l, :, :])
            nc.sync.dma_start(bs_sb[:, l:l + 1], bsin[l, :, :])
            if not trivial_affine:
                nc.sync.dma_start(grep_sb[:, l * H:(l + 1) * H], grep[l, :, :])
                nc.sync.dma_start(brep_sb[:, l * H:(l + 1) * H], brep[l, :, :])
        nc.sync.dma_start(ident_sb[:], ident[:, :])
        nc.sync.dma_start(identf_sb[:], identf[:, :])
        nc.sync.dma_start(gidx_sb[:], gidx[:, :])
        nc.vector.memset(eps_sb[:, :], LN_EPS)

        for s in range(SPC):
            # ---- h0 = emb, cast to fp16 during the DMA (SWDGE) ----------
            h16 = hp.tile([128, NT, H], F16, tag="h16")
            nc.vector.memset(h16[64:128, NT - 1, :], 0.0)
            for t in range(NT):
                r0 = t * 128
                pp = min(128, N - r0)
                nc.gpsimd.dma_start(h16[:pp, t, :], emb[s, r0:r0 + pp, :])

            # ---- selection + scatter + per-group gather -----------------
            wt = []
            for g in range(NGRP):
                wn = wnp.tile([128, NG, NP], F16, tag="wn")
                for q in range(NG):
                    t = g * NG + q
                    r0 = t * 128
                    pp = min(128, N - r0)
                    dt_ = dpool.tile([128, N], F32, tag="dt")
                    nc.sync.dma_start(dt_[:pp, :], dist[s, r0:r0 + pp, :])
                    # in-place negate: nd = -d
                    nc.scalar.activation(dt_[:pp, :], dt_[:pp, :], AF.Copy,
                                         bias=0.0, scale=-1.0)
                    m8 = sel.tile([128, 8], F32, tag="m8")
                    nc.vector.max(m8[:pp, :], dt_[:pp, :])
                    i16t = sel.tile([128, 8], U16, tag="i16")
                    nc.vector.max_index(i16t[:pp, :], m8[:pp, :], dt_[:pp, :])
                    # softmax over top-5 (shift-free: values in [-1, 0])
                    e5 = sel.tile([128, 5], F32, tag="e5")
                    z5 = sel.tile([128, 1], F32, tag="z5")
                    nc.scalar.activation(e5[:pp, :], m8[:pp, 0:5], AF.Exp,
                                         accum_out=z5[:pp, :])
                    r5 = sel.tile([128, 1], F32, tag="r5")
                    nc.vector.reciprocal(r5[:pp, :], z5[:pp, :])
                    w6 = sel.tile([128, 6], F16, tag="w6")
                    nc.vector.memset(w6[:pp, 5:6], 0.0)
                    nc.vector.tensor_scalar(w6[:pp, 0:5], e5[:pp, :], r5[:pp, :],
                                            None, OP.mult)
                    # zero the m-padding; scatter never writes it
                    nc.vector.memset(wn[:, q, N:NP], 0.0)
                    if pp < 128:
                        # zero rows the scatter won't touch (pad nodes)
                        base = pp // 32 * 32
                        nc.vector.memset(wn[base:128, q, :], 0.0)
                    # slot 5 carries the 6th-best index with weight 0.0
                    nc.gpsimd.local_scatter(
                        wn[0:pp, q, 0:N], w6[:pp, :],
                        i16t[:pp, 0:6].bitcast(I16),
                        channels=(pp + 15) // 16 * 16, num_elems=N, num_idxs=6)

                wtg = wtp.tile([128, NT, CHUNK], F16, tag=f"wt{g}")
                wt.append(wtg)
                nc.gpsimd.dma_gather(
                    out_ap=wtg[:, :, :],
                    in_ap=wn[:, :, :],
                    idxs_ap=gidx_sb[:, :],
                    num_idxs=CHUNK,
                    num_idxs_reg=CHUNK,
                    elem_size=NP,
                    transpose=True,
                    sbuf_tokens_per_rank=128,
                    sbuf_free_dim_per_rank=NP * 2,
                    queue_num=0,
                )

            # ---- layers --------------------------------------------------
            hout = None
            for l in range(L):
                # aggT[hd, n] = sum_m h16[m, hd] * w[n, m]
                aggT = agp.tile([128, NP], F16, tag="aggT")
                for ch in range(NGRP):
                    ps = ps_agg.tile([128, CHUNK], F32, tag="ps_aggT")
                    for c in range(NT):
                        nc.tensor.matmul(
                            ps[:, :], lhsT=h16[:, c, :],
                            rhs=wt[ch][:, c, :],
                            start=(c == 0), stop=(c == NT - 1))
                    nc.any.tensor_copy(aggT[:, ch * CHUNK:(ch + 1) * CHUNK],
                                       ps[:, :])
                # deltaT = relu(W @ aggT + b); f32 (PE transpose needs
                # out dtype == lhsT dtype and r accumulates in f32)
                deltaT = agp.tile([128, NP], F32, tag="deltaT")
                for ch in range(NGRP):
                    sl = slice(ch * CHUNK, (ch + 1) * CHUNK)
                    psd = ps_d.tile([128, CHUNK], F32, tag="psd")
                    nc.tensor.matmul(psd[:, :], lhsT=wsT_sb[:, l * H:(l + 1) * H],
                                     rhs=aggT[:, sl], start=True, stop=True)
                    nc.scalar.activation(deltaT[:, sl], psd[:, :], AF.Relu,
                                         bias=bs_sb[:, l:l + 1], scale=1.0)
                # r = delta^T^T + h  (transpose + identity-matmul residual).
                # PSUM flags grouped per 2KB bank: first matmul starts, last
                # stops; same-bank WAW keeps emission order on the PE stream.
                r = ps_r.tile([128, NP], F32, tag="r")
                SPB = 512 // 128  # 128-col slices per 2KB bank
                for g in range((NT + SPB - 1) // SPB):
                    ts_ = list(range(g * SPB, min((g + 1) * SPB, NT)))
                    for j, t in enumerate(ts_):
                        sl = slice(t * 128, (t + 1) * 128)
                        nc.tensor.matmul(r[:, sl], lhsT=deltaT[:, sl],
                                         rhs=identf_sb[:, :], is_transpose=True,
                                         start=(j == 0), stop=False)
                    for j, t in enumerate(ts_):
                        sl = slice(t * 128, (t + 1) * 128)
                        nc.tensor.matmul(r[:, sl], lhsT=ident_sb[:, :],
                                         rhs=h16[:, t, :], start=False,
                                         stop=(j == len(ts_) - 1))
                # layernorm stats
                sq = agp.tile([128, NP], F32, tag="sq")
                nc.scalar.square(sq[:, :], r[:, :])
                sr = ln.tile([128, NT], F32, tag="sr")
                nc.vector.tensor_reduce(
                    sr[:, :], r[:, :].rearrange("p (t h) -> p t h", h=H),
                    axis=mybir.AxisListType.X, op=OP.add)
                ssq = ln.tile([128, NT], F32, tag="ssq")
                nc.vector.tensor_reduce(
                    ssq[:, :], sq[:, :].rearrange("p (t h) -> p t h", h=H),
                    axis=mybir.AxisListType.X, op=OP.add)
                mu = ln.tile([128, NT], F32, tag="mu")
                nc.vector.tensor_scalar_mul(mu[:, :], sr[:, :], 1.0 / H)
                var = ln.tile([128, NT], F32, tag="var")
                # var = E[x^2] - mu^2  (+eps folded into sqrt bias)
                nc.vector.tensor_scalar_mul(var[:, :], ssq[:, :], 1.0 / H)
                musq = ln.tile([128, NT], F32, tag="musq")
                nc.vector.tensor_tensor(musq[:, :], mu[:, :], mu[:, :], OP.mult)
                nc.vector.tensor_tensor(var[:, :], var[:, :], musq[:, :],
                                        OP.subtract)
                sd = ln.tile([128, NT], F32, tag="sd")
                nc.scalar.activation(sd[:, :], var[:, :], AF.Sqrt,
                                     bias=eps_sb[:, :], scale=1.0)
                sinv = ln.tile([128, NT], F32, tag="sinv")
                nc.vector.reciprocal(sinv[:, :], sd[:, :])
                negmus = ln.tile([128, NT], F32, tag="negmus")
                nc.vector.tensor_tensor(negmus[:, :], mu[:, :], sinv[:, :],
                                        OP.mult)
                nc.vector.tensor_scalar_mul(negmus[:, :], negmus[:, :], -1.0)
                # normalize: h = (r - mu) * sinv  [* gamma + beta]
                last = l == L - 1
                if last:
                    hout = hp.tile([128, NT, H], F32, tag="hout")
                    dst = hout
                else:
                    dst = h16
                for t in range(NT):
                    sl = slice(t * 128, (t + 1) * 128)
                    if trivial_affine:
                        nc.any.tensor_scalar(dst[:, t, :], r[:, sl],
                                             sinv[:, t:t + 1],
                                             negmus[:, t:t + 1],
                                             OP.mult, OP.add)
                    else:
                        nc.any.tensor_scalar(sq[:, sl], r[:, sl],
                                             sinv[:, t:t + 1],
                                             negmus[:, t:t + 1],
                                             OP.mult, OP.add)
                        nc.vector.tensor_tensor(
                            sq[:, sl], sq[:, sl],
                            grep_sb[:, l * H:(l + 1) * H], OP.mult)
                        nc.vector.tensor_tensor(
                            dst[:, t, :], sq[:, sl],
                            brep_sb[:, l * H:(l + 1) * H], OP.add)

            # ---- store ---------------------------------------------------
            for t in range(NT):
                r0 = t * 128
                pp = min(128, N - r0)
                nc.sync.dma_start(out[s, r0:r0 + pp, :], hout[:pp, t, :])

    if compile:
        nc.compile()
    return nc


def _gather_idx_table():
    """Local index i of the gather lives at (partition i%16 + 16*core,
    free i//16), replicated across the 8 gpsimd cores; value = rank-local
    row (rank = i//128 selects the group tile's NG stripes)."""
    vals = np.arange(CHUNK, dtype=np.int16)
    tab = np.zeros((128, CHUNK // 16), dtype=np.int16)
    for g in range(8):
        tab[g * 16:(g + 1) * 16, :] = vals.reshape(CHUNK // 16, 16).T
    return tab


_CACHE = {}


def kernel(node_emb, dist_matrix, Ws, bs, gammas, betas):
    node_emb = np.ascontiguousarray(np.asarray(node_emb, dtype=np.float32))
    dist_matrix = np.ascontiguousarray(np.asarray(dist_matrix, dtype=np.float32))
    Ws = np.asarray(Ws, dtype=np.float32)
    bs = np.asarray(bs, dtype=np.float32)
    gammas = np.asarray(gammas, dtype=np.float32)
    betas = np.asarray(betas, dtype=np.float32)

    trivial = bool(np.all(gammas == 1.0) and np.all(betas == 0.0))
    key = ("prog", trivial)
    if key not in _CACHE:
        _CACHE[key] = _build_program(trivial)
    nc = _CACHE[key]

    wsT = np.ascontiguousarray(np.transpose(Ws, (0, 2, 1)).astype(np.float16))
    bs3 = np.ascontiguousarray(bs[:, :, None])
    ident = np.eye(H, dtype=np.float16)
    gtab = _gather_idx_table()

    in_maps = []
    for c in range(N_CORES):
        m = {
            "dist": dist_matrix[c * SPC:(c + 1) * SPC],
            "emb": node_emb[c * SPC:(c + 1) * SPC],
            "wsT": wsT,
            "bs": bs3,
            "ident": ident,
            "identf": np.eye(H, dtype=np.float32),
            "gidx": gtab,
        }
        if not trivial:
            m["grep"] = np.ascontiguousarray(
                np.broadcast_to(gammas[:, None, :], (L, H, H)))
            m["brep"] = np.ascontiguousarray(
                np.broadcast_to(betas[:, None, :], (L, H, H)))
        in_maps.append(m)

    res = run_bass_kernel_spmd(nc, in_maps, list(range(N_CORES)))
    kernel.last_results = res
    out = np.concatenate([res.results[c]["out"] for c in range(N_CORES)], axis=0)
    return out
